# revision 1
# baseline (speedup 1.0000x reference)
"""Trainium2 Bass kernel for nn_Attention_88441966559243.

Attention with additive bias [B,N,N] and per-key bool mask, fp32.
  B=2, N=2048, QD=1024, HEADS=16, DIM_HEAD=64.

Sharding: 8 cores = (batch b = core//4) x (query slice q0 = (core%4)*512).
Each core computes out[b, q0:q0+512, :] completely on-device; the host gather
is a pure concatenation. No collectives.

Per-core pipeline (all matmuls in float32r = full-rate ~tf32):
  A. transpose x on-chip via PE -> xT [1024,2048]; project qT (pre-scaled by
     1/8, SBUF-resident), kT -> DRAM scratch, and v' = [v | 1] (per-head ones
     column gives the softmax denominator for free) -> DRAM scratch.
  B. transpose the bias slice -> biasT [2048 keys, 512 queries], SBUF-resident.
  C. per head-pair (kT/v' slabs streamed back with full-partition, >=512B-row
     DMAs): sim^T chunks accumulated in PSUM on top of a PE-injected bias copy
     (identity matmul seeds has_written), masked exp in one ACT pass with the
     key mask as a per-partition bias vector, then U^T = v'^T @ e^T with the
     e@v matmuls lagged 6 chunks behind the exp producers (SBUF-buffered e^T)
     so the in-order PE queue never blocks on ACT.  Normalization via
     reciprocal_approx_fast of the PE-replicated denominator row.
  D. out = out_merged @ Wo + bo with per-head K=64 accumulation chunks.

Measured on HW (8 cores, For_i-loop timing): ~650 us/invocation,
rel err vs fp32 jax reference 4.4e-4 (float32r rounding).
"""
import sys
for _p in ("/opt/trn_rl_repo", "/root/.axon_site/_ro/trn_rl_repo"):
    if _p not in sys.path:
        sys.path.insert(0, _p)

import numpy as np

import concourse.bass as bass
import concourse.mybir as mybir
from concourse import bacc
from concourse.tile import TileContext
from concourse.masks import make_identity
from concourse.bass_utils import run_bass_kernel_spmd

F = 1024          # feature dim (QD == INNER)
NK = 2048         # keys (full sequence)
Q = 512           # queries per core
H = 16            # heads
D = 64            # head dim
DV = 65           # head dim + ones column
SCALE = D ** -0.5
MASK_NEG = -30000.0

FC = F // 128      # 8 feature chunks
KC = NK // 128     # 16 key chunks
NB = NK // 512     # 4 key 512-blocks

f32 = mybir.dt.float32
fr = mybir.dt.float32r
AF = mybir.ActivationFunctionType


import os


def build_nc(niter: int = 1, STAGES: str = "ABCD", ABL: str = ""):
    nc = bacc.Bacc(None, target_bir_lowering=False)

    x_in = nc.dram_tensor("x_in", [NK, F], f32, kind="ExternalInput")
    xq_in = nc.dram_tensor("xq_in", [Q, F], f32, kind="ExternalInput")
    bias_in = nc.dram_tensor("bias_in", [Q, NK], f32, kind="ExternalInput")
    maskneg_in = nc.dram_tensor("maskneg_in", [128, KC], f32, kind="ExternalInput")
    wq_in = nc.dram_tensor("wq_in", [F, F], fr, kind="ExternalInput")
    wkv_in = nc.dram_tensor("wkv_in", [F, 2 * F], fr, kind="ExternalInput")
    wo_in = nc.dram_tensor("wo_in", [F, F], fr, kind="ExternalInput")
    bo_in = nc.dram_tensor("bo_in", [1, F], fr, kind="ExternalInput")
    out_t = nc.dram_tensor("out_t", [Q, F], f32, kind="ExternalOutput")

    with TileContext(nc) as tc:
        with (
            tc.tile_pool(name="const", bufs=1) as constp,
            tc.tile_pool(name="dram", bufs=1, space="DRAM") as dramp,
            tc.tile_pool(name="ps", bufs=(4 if ABL == "quad" else 6), space="PSUM") as psA,
            tc.tile_pool(name="psu", bufs=(4 if ABL == "quad" else 2), space="PSUM") as psUp,
        ):
            # ---- constants ----
            ident = constp.tile([128, 128], f32)
            make_identity(nc, ident)
            ident_r = constp.tile([128, 128], fr)
            nc.scalar.copy(ident_r[:, :], ident[:, :])
            ident_q = constp.tile([128, 128], fr)
            nc.scalar.mul(ident_q[:, :], ident[:, :], 0.25)
            ones_f = constp.tile([128, 128], f32)
            nc.vector.memset(ones_f[:, :], 1.0)
            ones_r = constp.tile([128, 128], fr)
            nc.scalar.copy(ones_r[:, :], ones_f[:, :])
            masksb = constp.tile([128, KC], f32)
            nc.sync.dma_start(masksb[:, :], maskneg_in[:, :])
            bo_sb = constp.tile([1, F], fr)
            nc.sync.dma_start(bo_sb[:, :], bo_in[:, :])
            bo_rep = constp.tile([128, F], f32)

            vprime = dramp.tile([NK, H * DV], fr)       # v' (keys-major)
            kTd = dramp.tile([F, NK], fr)               # k^T
            outM = dramp.tile([F, Q], fr)               # merged out^T

            def body(_iv=None):
                with tc.tile_pool(name="qTp", bufs=1) as qTp:
                    qT = [qTp.tile([128, Q], fr, tag=f"qT{i}", name=f"qT{i}")
                          for i in range(FC)]

                    # ======== stage A ========
                    with (
                        tc.tile_pool(name="wload", bufs=8) as wlp,
                        tc.tile_pool(name="xTp", bufs=1) as xTp,
                        tc.tile_pool(name="kst", bufs=3) as kstp,
                    ):
                        xT = [xTp.tile([128, NK], fr, tag=f"xT{i}", name=f"xT{i}")
                              for i in range(FC)]

                        # A2 weights can load immediately
                        wq = [wlp.tile([128, F], fr, tag="w", name="w")
                              for _ in range(FC)]
                        for fc in range(FC):
                            nc.sync.dma_start(wq[fc][:, :],
                                              wq_in[fc * 128:(fc + 1) * 128, :])

                        with tc.tile_pool(name="xqTp", bufs=1) as xqTp:
                            xqT = [xqTp.tile([128, Q], fr, tag=f"xqT{i}",
                                             name=f"xqT{i}") for i in range(FC)]
                            # ---- A1: transpose x -> xT, xq -> xqT ----
                            with tc.tile_pool(name="xn", bufs=5) as xnp:
                                for rg in range(4):
                                    xns = []
                                    for r4 in range(4):
                                        rc = rg * 4 + r4
                                        xn = xnp.tile([128, F], f32, name="xn")
                                        nc.sync.dma_start(
                                            xn[:, :],
                                            x_in[rc * 128:(rc + 1) * 128, :])
                                        xns.append(xn)
                                    for fc in range(FC):
                                        ps = psA.tile([128, 512], f32, name="psa")
                                        for r4 in range(4):
                                            nc.tensor.transpose(
                                                ps[:, r4 * 128:(r4 + 1) * 128],
                                                xns[r4][:, fc * 128:(fc + 1) * 128],
                                                ident[:, :])
                                        nc.scalar.copy(
                                            xT[fc][:, rg * 512:(rg + 1) * 512],
                                            ps[:, :])
                                xqs = []
                                for r4 in range(4):
                                    xn = xnp.tile([128, F], f32, name="xn")
                                    nc.sync.dma_start(
                                        xn[:, :], xq_in[r4 * 128:(r4 + 1) * 128, :])
                                    xqs.append(xn)
                                for fc in range(FC):
                                    ps = psA.tile([128, 512], f32, name="psa")
                                    for r4 in range(4):
                                        nc.tensor.transpose(
                                            ps[:, r4 * 128:(r4 + 1) * 128],
                                            xqs[r4][:, fc * 128:(fc + 1) * 128],
                                            ident[:, :])
                                    nc.scalar.copy(xqT[fc][:, :], ps[:, :])

                            # ---- A2: qT = (Wq^T @ xqT) * SCALE ----
                            for m in range(FC):
                                ps = psA.tile([128, 512], f32, name="psa")
                                for fc in range(FC):
                                    nc.tensor.matmul(
                                        ps[:, :],
                                        wq[fc][:, m * 128:(m + 1) * 128],
                                        xqT[fc][:, :],
                                        start=(fc == 0), stop=(fc == FC - 1))
                                nc.scalar.mul(qT[m][:, :], ps[:, :], SCALE)

                        # ---- A3: kT = Wk^T @ xT -> DRAM ----
                        wk = [wlp.tile([128, F], fr, tag="w", name="w")
                              for _ in range(FC)]
                        for fc in range(FC):
                            nc.sync.dma_start(
                                wk[fc][:, :], wkv_in[fc * 128:(fc + 1) * 128, 0:F])
                        for m in range(FC):
                            kst = kstp.tile([128, NK], fr, name="kst")
                            for nb in range(NB):
                                ps = psA.tile([128, 512], f32, name="psa")
                                for fc in range(FC):
                                    nc.tensor.matmul(
                                        ps[:, :],
                                        wk[fc][:, m * 128:(m + 1) * 128],
                                        xT[fc][:, nb * 512:(nb + 1) * 512],
                                        start=(fc == 0), stop=(fc == FC - 1))
                                nc.scalar.copy(kst[:, nb * 512:(nb + 1) * 512],
                                               ps[:, :])
                            nc.sync.dma_start(kTd[m * 128:(m + 1) * 128, :],
                                              kst[:, :])

                        # ---- A4: v' = [x @ Wv | 1] -> DRAM ----
                        wv = [wlp.tile([128, F], fr, tag="w", name="w")
                              for _ in range(FC)]
                        for fc in range(FC):
                            nc.sync.dma_start(
                                wv[fc][:, :],
                                wkv_in[fc * 128:(fc + 1) * 128, F:2 * F])
                        with tc.tile_pool(name="vst", bufs=3) as vstp:
                            for kc in range(KC):
                                vst = vstp.tile([128, H * DV], fr, name="vst")
                                for half in range(2):
                                    ps = psA.tile([128, 512], f32, name="psa")
                                    for fc in range(FC):
                                        nc.tensor.matmul(
                                            ps[:, :],
                                            xT[fc][:, kc * 128:(kc + 1) * 128],
                                            wv[fc][:, half * 512:(half + 1) * 512],
                                            start=(fc == 0), stop=(fc == FC - 1))
                                    dst = vst[:, half * 8 * DV:(half + 1) * 8 * DV] \
                                        .rearrange("p (h x) -> p h x", x=DV)[:, :, 0:64]
                                    nc.scalar.copy(
                                        dst,
                                        ps[:, :].rearrange("p (h d) -> p h d", d=64))
                                ones_dst = vst[:, :].rearrange(
                                    "p (h x) -> p h x", x=DV)[:, :, 64:65]
                                nc.vector.tensor_copy(
                                    ones_dst,
                                    ones_r[:, 0:H].rearrange("p (a b) -> p a b", b=1))
                                nc.sync.dma_start(
                                    vprime[kc * 128:(kc + 1) * 128, :], vst[:, :])

                    if "B" not in STAGES:
                        with tc.tile_pool(name="dbg", bufs=2) as dbgp:
                            dbg = dbgp.tile([128, 512], fr, name="dbg")
                            nc.sync.dma_start(dbg[:, :], kTd[0:128, 0:512])
                            dbf = dbgp.tile([128, 512], f32, name="dbf")
                            nc.vector.tensor_copy(dbf[:, :], dbg[:, :])
                            nc.sync.dma_start(out_t[0:128, 0:512], dbf[:, :])
                        return

                    # ======== stages B + C ========
                    with tc.tile_pool(name="biasTp", bufs=1) as biasTp:
                        biasT = [biasTp.tile([128, Q], fr, tag=f"bT{i}",
                                             name=f"bT{i}") for i in range(KC)]
                        # ---- B: bias transpose ----
                        with tc.tile_pool(name="bn", bufs=4) as bnp:
                            bns = []
                            for qc in range(4):
                                bn = bnp.tile([128, NK], f32, name="bn")
                                nc.sync.dma_start(
                                    bn[:, :], bias_in[qc * 128:(qc + 1) * 128, :])
                                bns.append(bn)
                            for kc in range(KC):
                                ps = psA.tile([128, 512], f32, name="psa")
                                for qc in range(4):
                                    nc.tensor.transpose(
                                        ps[:, qc * 128:(qc + 1) * 128],
                                        bns[qc][:, kc * 128:(kc + 1) * 128],
                                        ident[:, :])
                                nc.scalar.copy(biasT[kc][:, :], ps[:, :])

                        if "C" not in STAGES:
                            with tc.tile_pool(name="dbg", bufs=2) as dbgp:
                                dbf = dbgp.tile([128, 512], f32, name="dbf")
                                nc.vector.tensor_copy(dbf[:, :], biasT[0][:, :])
                                nc.sync.dma_start(out_t[0:128, 0:512], dbf[:, :])
                            return

                        # ---- C: attention, head pairs ----
                        with (
                            tc.tile_pool(name="vph", bufs=2) as vphp,
                            tc.tile_pool(name="kph", bufs=2) as kphp,
                            tc.tile_pool(name="et", bufs=(12 if ABL == "quad" else 10)) as ep,
                            tc.tile_pool(name="dsb", bufs=2) as dsbp,
                            tc.tile_pool(name="rrep", bufs=2) as rrepp,
                            tc.tile_pool(name="otst", bufs=2) as otstp,
                        ):
                          if ABL == "quad":
                            for hq in range(H // 4):
                                vph = vphp.tile([128, KC * 4 * DV], fr, name="vph")
                                nc.sync.dma_start(
                                    vph[:, :].rearrange("p (kc d) -> p kc d",
                                                        d=4 * DV),
                                    vprime[:, 4 * hq * DV:(4 * hq + 4) * DV]
                                    .rearrange("(kc p) d -> p kc d", p=128))
                                kphs = []
                                for pp in range(2):
                                    kph = kphp.tile([128, NK], fr, name="kph")
                                    nc.sync.dma_start(
                                        kph[:, :],
                                        kTd[(2 * hq + pp) * 128:
                                            (2 * hq + pp + 1) * 128, :])
                                    kphs.append(kph)
                                psU4 = [psUp.tile([DV, 512], f32, name="psu")
                                        for _ in range(4)]
                                pending = []

                                def drain_av(upto):
                                    while pending and pending[0][0] <= upto:
                                        kc0, eTs = pending.pop(0)
                                        for sub in range(4):
                                            nc.tensor.matmul(
                                                psU4[sub][:, :],
                                                vph[:, kc0 * 4 * DV + sub * DV:
                                                    kc0 * 4 * DV + (sub + 1) * DV],
                                                eTs[sub][:, :],
                                                start=(kc0 == 0),
                                                stop=(kc0 == KC - 1))

                                for kc in range(KC):
                                    pss, eTs = [], []
                                    for sub in range(4):
                                        po = (sub % 2) * 64
                                        ps = psA.tile([128, 512], f32, name="psa")
                                        nc.tensor.matmul(ps[:, :], ident_r[:, :],
                                                         biasT[kc][:, :],
                                                         start=True, stop=False)
                                        nc.tensor.matmul(
                                            ps[:, :],
                                            kphs[sub // 2][po:po + 64,
                                                           kc * 128:(kc + 1) * 128],
                                            qT[2 * hq + sub // 2][po:po + 64, :],
                                            start=False, stop=True)
                                        pss.append(ps)
                                    for sub in range(4):
                                        eT = ep.tile([128, 512], fr, name="eT")
                                        nc.scalar.activation(
                                            eT[:, :], pss[sub][:, :], AF.Exp,
                                            bias=masksb[:, kc:kc + 1], scale=1.0)
                                        eTs.append(eT)
                                    pending.append((kc, eTs))
                                    drain_av(kc - 2)
                                drain_av(KC)
                                for sub in range(4):
                                    h = 4 * hq + sub
                                    psU = psU4[sub]
                                    Dsb = dsbp.tile([DV, 512], fr, name="Dsb")
                                    nc.scalar.copy(Dsb[64:65, :], psU[64:65, :])
                                    psR = psA.tile([128, 512], f32, name="psa")
                                    nc.tensor.matmul(psR[0:64, :],
                                                     ones_r[64:65, 0:64],
                                                     Dsb[64:65, :],
                                                     start=True, stop=True)
                                    rrep = rrepp.tile([64, 512], f32, name="rrep")
                                    nc.vector.reciprocal_approx_fast(
                                        out=rrep[:, :], in_=psR[0:64, :])
                                    ot = otstp.tile([64, Q], fr, name="ot")
                                    nc.vector.tensor_mul(ot[:, :], psU[0:64, :],
                                                         rrep[:, :])
                                    nc.sync.dma_start(
                                        outM[h * 64:(h + 1) * 64, :], ot[:, :])
                          else:
                            KCC = KC // 2 if ABL == "halfkc" else KC
                            hoisted = [None, None]
                            for hp in range(H // 2):
                                if ABL == "hoistdma" and hoisted[0] is not None:
                                    vph, kph = hoisted
                                else:
                                    # paired loads: full partitions, >=512B rows
                                    vph = vphp.tile([128, KC * 2 * DV], fr, name="vph")
                                    nc.sync.dma_start(
                                        vph[:, :].rearrange("p (kc d) -> p kc d",
                                                            d=2 * DV),
                                        vprime[:, 2 * hp * DV:(2 * hp + 2) * DV]
                                        .rearrange("(kc p) d -> p kc d", p=128))
                                    kph = kphp.tile([128, NK], fr, name="kph")
                                    nc.sync.dma_start(
                                        kph[:, :],
                                        kTd[hp * 128:(hp + 1) * 128, :])
                                    if ABL == "hoistdma":
                                        hoisted = [vph, kph]
                                psU2 = [psUp.tile([DV, 512], f32, name="psu")
                                        for _ in range(2)]
                                # lag e@v one chunk behind sim/exp so the
                                # in-order PE queue never waits on ACT.
                                pending = []

                                def drain_av(upto):
                                    while pending and pending[0][0] <= upto:
                                        kc0, eTs = pending.pop(0)
                                        for sub in range(2):
                                            nc.tensor.matmul(
                                                psU2[sub][:, :],
                                                vph[:, kc0 * 2 * DV + sub * DV:
                                                    kc0 * 2 * DV + (sub + 1) * DV],
                                                eTs[sub][:, :],
                                                start=(kc0 == 0),
                                                stop=(kc0 == KCC - 1))

                                for kc in range(KCC):
                                    pss, eTs = [], []
                                    for sub in range(2):
                                        po = sub * 64
                                        ps = psA.tile([128, 512], f32, name="psa")
                                        # bias injected as 4 quarter-
                                        # strength identity matmuls: the extra
                                        # PE passes are redundant FLOP-wise
                                        # but keep enough PE work in flight
                                        # per PSUM slot to hide the producer->
                                        # consumer round-trip latency.
                                        for ks in range(4):
                                            nc.tensor.matmul(
                                                ps[:, :], ident_q[:, :],
                                                biasT[kc][:, :],
                                                start=(ks == 0), stop=False)
                                        nc.tensor.matmul(
                                            ps[:, :],
                                            kph[po:po + 64,
                                                kc * 128:(kc + 1) * 128],
                                            qT[hp][po:po + 64, :],
                                            start=False, stop=True)
                                        pss.append(ps)
                                    for sub in range(2):
                                        eT = ep.tile([128, 512], fr, name="eT")
                                        if ABL == "dvecopy":
                                            nc.vector.tensor_copy(eT[:, :],
                                                                  pss[sub][:, :])
                                        elif ABL == "nobias":
                                            nc.scalar.activation(
                                                eT[:, :], pss[sub][:, :], AF.Exp,
                                                scale=1.0)
                                        else:
                                            nc.scalar.activation(
                                                eT[:, :], pss[sub][:, :], AF.Exp,
                                                bias=masksb[:, kc:kc + 1], scale=1.0)
                                        eTs.append(eT)
                                    pending.append((kc, eTs))
                                    # batched drain: one PE wait covers 4 avs (ACT in-order)
                                    if kc >= 9 and (kc - 9) % 4 == 3:
                                        drain_av(kc - 6)
                                drain_av(KCC)
                                for sub in range(2):
                                    h = 2 * hp + sub
                                    psU = psU2[sub]
                                    Dsb = dsbp.tile([DV, 512], fr, name="Dsb")
                                    nc.scalar.copy(Dsb[64:65, :], psU[64:65, :])
                                    psR = psA.tile([128, 512], f32, name="psa")
                                    nc.tensor.matmul(psR[0:64, :],
                                                     ones_r[64:65, 0:64],
                                                     Dsb[64:65, :],
                                                     start=True, stop=True)
                                    rrep = rrepp.tile([64, 512], f32, name="rrep")
                                    nc.vector.reciprocal_approx_fast(
                                        out=rrep[:, :], in_=psR[0:64, :])
                                    ot = otstp.tile([64, Q], fr, name="ot")
                                    nc.vector.tensor_mul(ot[:, :], psU[0:64, :],
                                                         rrep[:, :])
                                    nc.sync.dma_start(
                                        outM[h * 64:(h + 1) * 64, :], ot[:, :])

                if "D" not in STAGES:
                    with tc.tile_pool(name="dbg", bufs=2) as dbgp:
                        dbg = dbgp.tile([128, 512], fr, name="dbg")
                        nc.sync.dma_start(dbg[:, :], outM[0:128, :])
                        dbf = dbgp.tile([128, 512], f32, name="dbf")
                        nc.vector.tensor_copy(dbf[:, :], dbg[:, :])
                        nc.sync.dma_start(out_t[0:128, 0:512], dbf[:, :])
                    return

                # ======== stage D ========
                with (
                    tc.tile_pool(name="wop", bufs=1) as wop,
                    tc.tile_pool(name="oMp", bufs=1) as oMp,
                    tc.tile_pool(name="fin", bufs=3) as finp,
                ):
                    wo = [wop.tile([64, F], fr, tag=f"wo{i}", name=f"wo{i}")
                          for i in range(H)]
                    oM = [oMp.tile([64, Q], fr, tag=f"oM{i}", name=f"oM{i}")
                          for i in range(H)]
                    for i in range(H):
                        nc.sync.dma_start(wo[i][:, :],
                                          wo_in[i * 64:(i + 1) * 64, :])
                        nc.sync.dma_start(oM[i][:, :],
                                          outM[i * 64:(i + 1) * 64, :])
                    for nb2 in range(2):
                        ps = psA.tile([128, 512], f32, name="psa")
                        nc.tensor.matmul(ps[:, :], ones_r[0:1, 0:128],
                                         bo_sb[0:1, nb2 * 512:(nb2 + 1) * 512],
                                         start=True, stop=True)
                        nc.scalar.copy(bo_rep[:, nb2 * 512:(nb2 + 1) * 512],
                                       ps[:, :])
                    if STAGES == "ABCD1":
                        dbf = finp.tile([128, 512], f32, name="fin")
                        nc.vector.tensor_copy(dbf[:, :], bo_rep[:, 0:512])
                        nc.sync.dma_start(out_t[0:128, 0:512], dbf[:, :])
                        return
                    nheads = 2 if STAGES == "ABCD2" else H
                    for mc in range(4):
                        for nb2 in range(2):
                            psF = psA.tile([128, 512], f32, name="psa")
                            for h in range(nheads):
                                nc.tensor.matmul(
                                    psF[:, :],
                                    oM[h][:, mc * 128:(mc + 1) * 128],
                                    wo[h][:, nb2 * 512:(nb2 + 1) * 512],
                                    start=(h == 0), stop=(h == nheads - 1))
                            fin = finp.tile([128, 512], f32, name="fin")
                            nc.vector.tensor_add(
                                fin[:, :], psF[:, :],
                                bo_rep[:, nb2 * 512:(nb2 + 1) * 512])
                            nc.sync.dma_start(
                                out_t[mc * 128:(mc + 1) * 128,
                                      nb2 * 512:(nb2 + 1) * 512],
                                fin[:, :])

            if niter == 1:
                body()
            else:
                with tc.For_i(0, niter, 1) as iv:
                    body(iv)

    nc.finalize()
    return nc


_nc_cache = {}


def _get_nc(niter=1):
    if niter not in _nc_cache:
        _nc_cache[niter] = build_nc(niter)
    return _nc_cache[niter]


def make_in_maps(x, bias, mask, Wq, Wkv, Wo, bo):
    x = np.asarray(x, dtype=np.float32)
    bias = np.asarray(bias, dtype=np.float32)
    mask = np.asarray(mask)
    in_maps = []
    for c in range(8):
        b, qi = c // 4, c % 4
        q0 = qi * Q
        maskneg = np.where(mask[b], 0.0, MASK_NEG).astype(np.float32)
        in_maps.append({
            "x_in": np.ascontiguousarray(x[b]),
            "xq_in": np.ascontiguousarray(x[b, q0:q0 + Q]),
            "bias_in": np.ascontiguousarray(bias[b, q0:q0 + Q]),
            "maskneg_in": np.ascontiguousarray(maskneg.reshape(KC, 128).T),
            "wq_in": np.ascontiguousarray(np.asarray(Wq, dtype=np.float32)),
            "wkv_in": np.ascontiguousarray(np.asarray(Wkv, dtype=np.float32)),
            "wo_in": np.ascontiguousarray(np.asarray(Wo, dtype=np.float32)),
            "bo_in": np.ascontiguousarray(
                np.asarray(bo, dtype=np.float32).reshape(1, F)),
        })
    return in_maps


class _CachedRunner:
    """Jit the NEFF-backed executable once; repeat kernel() calls then skip
    the ~40s relower/recompile and run in ~0.1s."""

    def __init__(self, nc, n_cores=8):
        import jax
        from jax.sharding import Mesh, PartitionSpec
        from jax.experimental.shard_map import shard_map
        from concourse.bass2jax import (_bass_exec_p, install_neuronx_cc_hook,
                                        partition_id_tensor)
        install_neuronx_cc_hook()
        self.jax = jax
        self.n_cores = n_cores
        pname = nc.partition_id_tensor.name if nc.partition_id_tensor else None
        in_names, out_names, out_avals, zeros = [], [], [], []
        for alloc in nc.m.functions[0].allocations:
            if not isinstance(alloc, mybir.MemoryLocationSet):
                continue
            name = alloc.memorylocations[0].name
            if alloc.kind == "ExternalInput":
                if name != pname:
                    in_names.append(name)
            elif alloc.kind == "ExternalOutput":
                out_names.append(name)
                shape = tuple(alloc.tensor_shape)
                dt_np = mybir.dt.np(alloc.dtype)
                out_avals.append(jax.core.ShapedArray(shape, dt_np))
                zeros.append(np.zeros(shape, dt_np))
        self.in_names, self.out_names = in_names, out_names
        self.out_avals, self.zeros = out_avals, zeros
        all_names = in_names + out_names + ([pname] if pname else [])

        def _body(*args):
            ops = list(args)
            if pname is not None:
                ops.append(partition_id_tensor())
            return tuple(_bass_exec_p.bind(
                *ops, out_avals=tuple(out_avals), in_names=tuple(all_names),
                out_names=tuple(out_names), lowering_input_output_aliases=(),
                sim_require_finite=True, sim_require_nnan=True, nc=nc))

        mesh = Mesh(np.asarray(jax.devices()[:n_cores]), ("core",))
        spec_in = (PartitionSpec("core"),) * (len(in_names) + len(out_names))
        spec_out = (PartitionSpec("core"),) * len(out_names)
        self.fn = jax.jit(shard_map(_body, mesh=mesh, in_specs=spec_in,
                                    out_specs=spec_out, check_rep=False),
                          keep_unused=True)

    def run(self, in_maps):
        n = self.n_cores
        args = [np.concatenate([np.asarray(in_maps[c][k]) for c in range(n)], axis=0)
                for k in self.in_names]
        args += [np.zeros((n * z.shape[0], *z.shape[1:]), z.dtype)
                 for z in self.zeros]
        outs = self.fn(*args)
        self.jax.block_until_ready(outs)
        return [{k: np.asarray(outs[i]).reshape(n, *self.out_avals[i].shape)[c]
                 for i, k in enumerate(self.out_names)} for c in range(n)]


_runner_cache = {}


def kernel(x, bias, mask, Wq, Wkv, Wo, bo):
    in_maps = make_in_maps(x, bias, mask, Wq, Wkv, Wo, bo)
    try:
        if "r" not in _runner_cache:
            _runner_cache["r"] = _CachedRunner(_get_nc(1))
        results = _runner_cache["r"].run(in_maps)
    except Exception:
        _runner_cache.pop("r", None)
        res = run_bass_kernel_spmd(_get_nc(1), in_maps, core_ids=list(range(8)))
        results = res.results
    out = np.empty((2, NK, F), dtype=np.float32)
    for c in range(8):
        b, qi = c // 4, c % 4
        out[b, qi * Q:(qi + 1) * Q] = results[c]["out_t"]
    return out



# revision 2
# speedup vs baseline: 1.2727x; 1.2727x over previous
"""Trainium2 Bass kernel for nn_Attention_88441966559243.

Attention with additive bias [B,N,N] and per-key bool mask, fp32.
  B=2, N=2048, QD=1024, HEADS=16, DIM_HEAD=64.

Sharding: 8 cores = (batch b = core//4) x (query slice q0 = (core%4)*512).
Each core computes out[b, q0:q0+512, :] completely on-device; the host gather
is a pure concatenation. No collectives.

Per-core pipeline (all matmuls in float32r = full-rate ~tf32):
  A. transpose x on-chip via PE -> xT [1024,2048]; project qT (pre-scaled by
     1/8, SBUF-resident), kT -> DRAM scratch, and v' = [v | 1] (per-head ones
     column gives the softmax denominator for free) -> DRAM scratch.
  B. transpose the bias slice and exponentiate with the key mask folded in:
     EB[key, q] = exp(biasT + maskneg), SBUF-resident.  exp(s+b+m) is then
     exp(s) * EB -- no PSUM bias seeding needed, saving 4 identity matmuls
     per sim chunk on the PE.
  C. per head-pair (kT/v' slabs streamed back with full-partition, >=512B-row
     DMAs): sim^T chunks in PSUM (single matmul, start&stop), exp on ACT,
     EB multiply on DVE, then U^T = v'^T @ e^T with the e@v matmuls lagged 6
     chunks behind so the in-order PE queue never blocks on ACT/DVE.
     Normalization via reciprocal_approx_fast of the PE-replicated
     denominator row, written straight into packed [128,Q] head-pair tiles.
  D. out = oPair @ Wo + bo with per-pair K=128 accumulation chunks (SBUF
     resident, no DRAM round-trip).
"""
import sys
for _p in ("/opt/trn_rl_repo", "/root/.axon_site/_ro/trn_rl_repo"):
    if _p not in sys.path:
        sys.path.insert(0, _p)

import numpy as np

import concourse.bass as bass
import concourse.mybir as mybir
from concourse import bacc
from concourse.tile import TileContext
from concourse.masks import make_identity
from concourse.bass_utils import run_bass_kernel_spmd

F = 1024          # feature dim (QD == INNER)
NK = 2048         # keys (full sequence)
Q = 512           # queries per core
H = 16            # heads
D = 64            # head dim
DV = 65           # head dim + ones column
SCALE = D ** -0.5
MASK_NEG = -30000.0

FC = F // 128      # 8 feature chunks
KC = NK // 128     # 16 key chunks
NB = NK // 512     # 4 key 512-blocks

f32 = mybir.dt.float32
fr = mybir.dt.float32r
AF = mybir.ActivationFunctionType


def build_nc(niter: int = 1):
    nc = bacc.Bacc(None, target_bir_lowering=False)

    x_in = nc.dram_tensor("x_in", [NK, F], f32, kind="ExternalInput")
    xq_in = nc.dram_tensor("xq_in", [Q, F], f32, kind="ExternalInput")
    bias_in = nc.dram_tensor("bias_in", [Q, NK], f32, kind="ExternalInput")
    maskneg_in = nc.dram_tensor("maskneg_in", [128, KC], f32, kind="ExternalInput")
    wq_in = nc.dram_tensor("wq_in", [F, F], fr, kind="ExternalInput")
    wkv_in = nc.dram_tensor("wkv_in", [F, 2 * F], fr, kind="ExternalInput")
    wo_in = nc.dram_tensor("wo_in", [F, F], fr, kind="ExternalInput")
    bo_in = nc.dram_tensor("bo_in", [1, F], fr, kind="ExternalInput")
    out_t = nc.dram_tensor("out_t", [Q, F], f32, kind="ExternalOutput")

    with TileContext(nc) as tc:
        with (
            tc.tile_pool(name="const", bufs=1) as constp,
            tc.tile_pool(name="dram", bufs=1, space="DRAM") as dramp,
            tc.tile_pool(name="ps", bufs=6, space="PSUM") as psA,
            tc.tile_pool(name="psu", bufs=2, space="PSUM") as psUp,
        ):
            # ---- constants ----
            ident = constp.tile([128, 128], f32)
            make_identity(nc, ident)
            ones_f = constp.tile([128, 128], f32)
            nc.vector.memset(ones_f[:, :], 1.0)
            ones_r = constp.tile([128, 128], fr)
            nc.scalar.copy(ones_r[:, :], ones_f[:, :])
            masksb = constp.tile([128, KC], f32)
            nc.sync.dma_start(masksb[:, :], maskneg_in[:, :])
            bo_sb = constp.tile([1, F], fr)
            nc.sync.dma_start(bo_sb[:, :], bo_in[:, :])
            bo_rep = constp.tile([128, F], f32)

            vprime = dramp.tile([NK, H * DV], fr)       # v' (keys-major)
            kTd = dramp.tile([F, NK], fr)               # k^T

            def body(_iv=None):
                with tc.tile_pool(name="qTp", bufs=1) as qTp:
                    qT = [qTp.tile([128, Q], fr, tag=f"qT{i}", name=f"qT{i}")
                          for i in range(FC)]

                    # ======== stage A ========
                    with (
                        tc.tile_pool(name="wload", bufs=8) as wlp,
                        tc.tile_pool(name="xTp", bufs=1) as xTp,
                        tc.tile_pool(name="kst", bufs=3) as kstp,
                    ):
                        xT = [xTp.tile([128, NK], fr, tag=f"xT{i}", name=f"xT{i}")
                              for i in range(FC)]

                        # A2 weights can load immediately
                        wq = [wlp.tile([128, F], fr, tag="w", name="w")
                              for _ in range(FC)]
                        for fc in range(FC):
                            nc.sync.dma_start(wq[fc][:, :],
                                              wq_in[fc * 128:(fc + 1) * 128, :])

                        with tc.tile_pool(name="xqTp", bufs=1) as xqTp:
                            xqT = [xqTp.tile([128, Q], fr, tag=f"xqT{i}",
                                             name=f"xqT{i}") for i in range(FC)]
                            # ---- A1: transpose x -> xT, xq -> xqT ----
                            with tc.tile_pool(name="xn", bufs=5) as xnp:
                                for rg in range(4):
                                    xns = []
                                    for r4 in range(4):
                                        rc = rg * 4 + r4
                                        xn = xnp.tile([128, F], f32, name="xn")
                                        nc.sync.dma_start(
                                            xn[:, :],
                                            x_in[rc * 128:(rc + 1) * 128, :])
                                        xns.append(xn)
                                    for fc in range(FC):
                                        ps = psA.tile([128, 512], f32, name="psa")
                                        for r4 in range(4):
                                            nc.tensor.transpose(
                                                ps[:, r4 * 128:(r4 + 1) * 128],
                                                xns[r4][:, fc * 128:(fc + 1) * 128],
                                                ident[:, :])
                                        nc.scalar.copy(
                                            xT[fc][:, rg * 512:(rg + 1) * 512],
                                            ps[:, :])
                                xqs = []
                                for r4 in range(4):
                                    xn = xnp.tile([128, F], f32, name="xn")
                                    nc.sync.dma_start(
                                        xn[:, :], xq_in[r4 * 128:(r4 + 1) * 128, :])
                                    xqs.append(xn)
                                for fc in range(FC):
                                    ps = psA.tile([128, 512], f32, name="psa")
                                    for r4 in range(4):
                                        nc.tensor.transpose(
                                            ps[:, r4 * 128:(r4 + 1) * 128],
                                            xqs[r4][:, fc * 128:(fc + 1) * 128],
                                            ident[:, :])
                                    nc.scalar.copy(xqT[fc][:, :], ps[:, :])

                            # ---- A2: qT = (Wq^T @ xqT) * SCALE ----
                            for m in range(FC):
                                ps = psA.tile([128, 512], f32, name="psa")
                                for fc in range(FC):
                                    nc.tensor.matmul(
                                        ps[:, :],
                                        wq[fc][:, m * 128:(m + 1) * 128],
                                        xqT[fc][:, :],
                                        start=(fc == 0), stop=(fc == FC - 1))
                                nc.scalar.mul(qT[m][:, :], ps[:, :], SCALE)

                        # ---- A3: kT = Wk^T @ xT -> DRAM ----
                        wk = [wlp.tile([128, F], fr, tag="w", name="w")
                              for _ in range(FC)]
                        for fc in range(FC):
                            nc.sync.dma_start(
                                wk[fc][:, :], wkv_in[fc * 128:(fc + 1) * 128, 0:F])
                        for m in range(FC):
                            kst = kstp.tile([128, NK], fr, name="kst")
                            for nb in range(NB):
                                ps = psA.tile([128, 512], f32, name="psa")
                                for fc in range(FC):
                                    nc.tensor.matmul(
                                        ps[:, :],
                                        wk[fc][:, m * 128:(m + 1) * 128],
                                        xT[fc][:, nb * 512:(nb + 1) * 512],
                                        start=(fc == 0), stop=(fc == FC - 1))
                                nc.scalar.copy(kst[:, nb * 512:(nb + 1) * 512],
                                               ps[:, :])
                            nc.sync.dma_start(kTd[m * 128:(m + 1) * 128, :],
                                              kst[:, :])

                        # ---- A4: v' = [x @ Wv | 1] -> DRAM ----
                        wv = [wlp.tile([128, F], fr, tag="w", name="w")
                              for _ in range(FC)]
                        for fc in range(FC):
                            nc.sync.dma_start(
                                wv[fc][:, :],
                                wkv_in[fc * 128:(fc + 1) * 128, F:2 * F])
                        with tc.tile_pool(name="vst", bufs=3) as vstp:
                            for kc in range(KC):
                                vst = vstp.tile([128, H * DV], fr, name="vst")
                                for half in range(2):
                                    ps = psA.tile([128, 512], f32, name="psa")
                                    for fc in range(FC):
                                        nc.tensor.matmul(
                                            ps[:, :],
                                            xT[fc][:, kc * 128:(kc + 1) * 128],
                                            wv[fc][:, half * 512:(half + 1) * 512],
                                            start=(fc == 0), stop=(fc == FC - 1))
                                    dst = vst[:, half * 8 * DV:(half + 1) * 8 * DV] \
                                        .rearrange("p (h x) -> p h x", x=DV)[:, :, 0:64]
                                    nc.scalar.copy(
                                        dst,
                                        ps[:, :].rearrange("p (h d) -> p h d", d=64))
                                ones_dst = vst[:, :].rearrange(
                                    "p (h x) -> p h x", x=DV)[:, :, 64:65]
                                nc.vector.tensor_copy(
                                    ones_dst,
                                    ones_r[:, 0:H].rearrange("p (a b) -> p a b", b=1))
                                nc.sync.dma_start(
                                    vprime[kc * 128:(kc + 1) * 128, :], vst[:, :])

                    # ======== stages B + C ========
                    with (
                        tc.tile_pool(name="EBp", bufs=1) as EBp,
                        tc.tile_pool(name="oPp", bufs=1) as oPp,
                    ):
                        EB = [EBp.tile([128, Q], f32, tag=f"EB{i}",
                                       name=f"EB{i}") for i in range(KC)]
                        oPair = [oPp.tile([128, Q], fr, tag=f"oP{i}",
                                          name=f"oP{i}") for i in range(H // 2)]
                        # ---- B: EB = exp(bias^T + maskneg) ----
                        with tc.tile_pool(name="bn", bufs=4) as bnp:
                            bns = []
                            for qc in range(4):
                                bn = bnp.tile([128, NK], f32, name="bn")
                                nc.sync.dma_start(
                                    bn[:, :], bias_in[qc * 128:(qc + 1) * 128, :])
                                bns.append(bn)
                            for kc in range(KC):
                                ps = psA.tile([128, 512], f32, name="psa")
                                for qc in range(4):
                                    nc.tensor.transpose(
                                        ps[:, qc * 128:(qc + 1) * 128],
                                        bns[qc][:, kc * 128:(kc + 1) * 128],
                                        ident[:, :])
                                nc.scalar.activation(
                                    EB[kc][:, :], ps[:, :], AF.Exp,
                                    bias=masksb[:, kc:kc + 1], scale=1.0)

                        # ---- C: attention, head pairs ----
                        with (
                            tc.tile_pool(name="vph", bufs=2) as vphp,
                            tc.tile_pool(name="kph", bufs=2) as kphp,
                            tc.tile_pool(name="ef", bufs=6) as efp,
                            tc.tile_pool(name="et", bufs=16) as ep,
                            tc.tile_pool(name="dsb", bufs=2) as dsbp,
                            tc.tile_pool(name="rrep", bufs=2) as rrepp,
                        ):
                            for hp in range(H // 2):
                                # paired loads: full partitions, >=512B rows
                                vph = vphp.tile([128, KC * 2 * DV], fr, name="vph")
                                nc.sync.dma_start(
                                    vph[:, :].rearrange("p (kc d) -> p kc d",
                                                        d=2 * DV),
                                    vprime[:, 2 * hp * DV:(2 * hp + 2) * DV]
                                    .rearrange("(kc p) d -> p kc d", p=128))
                                kph = kphp.tile([128, NK], fr, name="kph")
                                nc.sync.dma_start(
                                    kph[:, :],
                                    kTd[hp * 128:(hp + 1) * 128, :])
                                psU2 = [psUp.tile([DV, 512], f32, name="psu")
                                        for _ in range(2)]
                                # lag e@v chunks behind sim/exp/mul so the
                                # in-order PE queue never waits on ACT/DVE.
                                pending = []

                                def drain_av(upto):
                                    while pending and pending[0][0] <= upto:
                                        kc0, eTs = pending.pop(0)
                                        for sub in range(2):
                                            nc.tensor.matmul(
                                                psU2[sub][:, :],
                                                vph[:, kc0 * 2 * DV + sub * DV:
                                                    kc0 * 2 * DV + (sub + 1) * DV],
                                                eTs[sub][:, :],
                                                start=(kc0 == 0),
                                                stop=(kc0 == KC - 1))

                                for kc in range(KC):
                                    pss, eTs = [], []
                                    for sub in range(2):
                                        po = sub * 64
                                        ps = psA.tile([128, 512], f32, name="psa")
                                        nc.tensor.matmul(
                                            ps[:, :],
                                            kph[po:po + 64,
                                                kc * 128:(kc + 1) * 128],
                                            qT[hp][po:po + 64, :],
                                            start=True, stop=True)
                                        pss.append(ps)
                                    for sub in range(2):
                                        ef = efp.tile([128, 512], f32, name="ef")
                                        nc.scalar.activation(
                                            ef[:, :], pss[sub][:, :], AF.Exp,
                                            scale=1.0)
                                        eT = ep.tile([128, 512], fr, name="eT")
                                        nc.vector.tensor_mul(eT[:, :], ef[:, :],
                                                             EB[kc][:, :])
                                        eTs.append(eT)
                                    pending.append((kc, eTs))
                                    drain_av(kc - 6)
                                drain_av(KC)
                                for sub in range(2):
                                    psU = psU2[sub]
                                    Dsb = dsbp.tile([DV, 512], fr, name="Dsb")
                                    nc.scalar.copy(Dsb[64:65, :], psU[64:65, :])
                                    psR = psA.tile([128, 512], f32, name="psa")
                                    nc.tensor.matmul(psR[0:64, :],
                                                     ones_r[64:65, 0:64],
                                                     Dsb[64:65, :],
                                                     start=True, stop=True)
                                    rrep = rrepp.tile([64, 512], f32, name="rrep")
                                    nc.vector.reciprocal_approx_fast(
                                        out=rrep[:, :], in_=psR[0:64, :])
                                    nc.vector.tensor_mul(
                                        oPair[hp][sub * 64:(sub + 1) * 64, :],
                                        psU[0:64, :], rrep[:, :])

                        # ======== stage D ========
                        with (
                            tc.tile_pool(name="wop", bufs=1) as wop,
                            tc.tile_pool(name="fin", bufs=3) as finp,
                        ):
                            wo = [wop.tile([128, F], fr, tag=f"wo{i}",
                                           name=f"wo{i}") for i in range(H // 2)]
                            for i in range(H // 2):
                                nc.sync.dma_start(
                                    wo[i][:, :], wo_in[i * 128:(i + 1) * 128, :])
                            for nb2 in range(2):
                                ps = psA.tile([128, 512], f32, name="psa")
                                nc.tensor.matmul(
                                    ps[:, :], ones_r[0:1, 0:128],
                                    bo_sb[0:1, nb2 * 512:(nb2 + 1) * 512],
                                    start=True, stop=True)
                                nc.scalar.copy(
                                    bo_rep[:, nb2 * 512:(nb2 + 1) * 512],
                                    ps[:, :])
                            for mc in range(4):
                                for nb2 in range(2):
                                    psF = psA.tile([128, 512], f32, name="psa")
                                    for hp in range(H // 2):
                                        nc.tensor.matmul(
                                            psF[:, :],
                                            oPair[hp][:, mc * 128:(mc + 1) * 128],
                                            wo[hp][:, nb2 * 512:(nb2 + 1) * 512],
                                            start=(hp == 0),
                                            stop=(hp == H // 2 - 1))
                                    fin = finp.tile([128, 512], f32, name="fin")
                                    nc.vector.tensor_add(
                                        fin[:, :], psF[:, :],
                                        bo_rep[:, nb2 * 512:(nb2 + 1) * 512])
                                    nc.sync.dma_start(
                                        out_t[mc * 128:(mc + 1) * 128,
                                              nb2 * 512:(nb2 + 1) * 512],
                                        fin[:, :])

            if niter == 1:
                body()
            else:
                with tc.For_i(0, niter, 1) as iv:
                    body(iv)

    nc.finalize()
    return nc


_nc_cache = {}


def _get_nc(niter=1):
    if niter not in _nc_cache:
        _nc_cache[niter] = build_nc(niter)
    return _nc_cache[niter]


def make_in_maps(x, bias, mask, Wq, Wkv, Wo, bo):
    x = np.asarray(x, dtype=np.float32)
    bias = np.asarray(bias, dtype=np.float32)
    mask = np.asarray(mask)
    in_maps = []
    for c in range(8):
        b, qi = c // 4, c % 4
        q0 = qi * Q
        maskneg = np.where(mask[b], 0.0, MASK_NEG).astype(np.float32)
        in_maps.append({
            "x_in": np.ascontiguousarray(x[b]),
            "xq_in": np.ascontiguousarray(x[b, q0:q0 + Q]),
            "bias_in": np.ascontiguousarray(bias[b, q0:q0 + Q]),
            "maskneg_in": np.ascontiguousarray(maskneg.reshape(KC, 128).T),
            "wq_in": np.ascontiguousarray(np.asarray(Wq, dtype=np.float32)),
            "wkv_in": np.ascontiguousarray(np.asarray(Wkv, dtype=np.float32)),
            "wo_in": np.ascontiguousarray(np.asarray(Wo, dtype=np.float32)),
            "bo_in": np.ascontiguousarray(
                np.asarray(bo, dtype=np.float32).reshape(1, F)),
        })
    return in_maps


class _CachedRunner:
    """Jit the NEFF-backed executable once; repeat kernel() calls then skip
    the ~40s relower/recompile and run in ~0.1s."""

    def __init__(self, nc, n_cores=8):
        import jax
        from jax.sharding import Mesh, PartitionSpec
        from jax.experimental.shard_map import shard_map
        from concourse.bass2jax import (_bass_exec_p, install_neuronx_cc_hook,
                                        partition_id_tensor)
        install_neuronx_cc_hook()
        self.jax = jax
        self.n_cores = n_cores
        pname = nc.partition_id_tensor.name if nc.partition_id_tensor else None
        in_names, out_names, out_avals, zeros = [], [], [], []
        for alloc in nc.m.functions[0].allocations:
            if not isinstance(alloc, mybir.MemoryLocationSet):
                continue
            name = alloc.memorylocations[0].name
            if alloc.kind == "ExternalInput":
                if name != pname:
                    in_names.append(name)
            elif alloc.kind == "ExternalOutput":
                out_names.append(name)
                shape = tuple(alloc.tensor_shape)
                dt_np = mybir.dt.np(alloc.dtype)
                out_avals.append(jax.core.ShapedArray(shape, dt_np))
                zeros.append(np.zeros(shape, dt_np))
        self.in_names, self.out_names = in_names, out_names
        self.out_avals, self.zeros = out_avals, zeros
        all_names = in_names + out_names + ([pname] if pname else [])

        def _body(*args):
            ops = list(args)
            if pname is not None:
                ops.append(partition_id_tensor())
            return tuple(_bass_exec_p.bind(
                *ops, out_avals=tuple(out_avals), in_names=tuple(all_names),
                out_names=tuple(out_names), lowering_input_output_aliases=(),
                sim_require_finite=True, sim_require_nnan=True, nc=nc))

        mesh = Mesh(np.asarray(jax.devices()[:n_cores]), ("core",))
        spec_in = (PartitionSpec("core"),) * (len(in_names) + len(out_names))
        spec_out = (PartitionSpec("core"),) * len(out_names)
        self.fn = jax.jit(shard_map(_body, mesh=mesh, in_specs=spec_in,
                                    out_specs=spec_out, check_rep=False),
                          keep_unused=True)

    def run(self, in_maps):
        n = self.n_cores
        args = [np.concatenate([np.asarray(in_maps[c][k]) for c in range(n)], axis=0)
                for k in self.in_names]
        args += [np.zeros((n * z.shape[0], *z.shape[1:]), z.dtype)
                 for z in self.zeros]
        outs = self.fn(*args)
        self.jax.block_until_ready(outs)
        return [{k: np.asarray(outs[i]).reshape(n, *self.out_avals[i].shape)[c]
                 for i, k in enumerate(self.out_names)} for c in range(n)]


_runner_cache = {}


def kernel(x, bias, mask, Wq, Wkv, Wo, bo):
    in_maps = make_in_maps(x, bias, mask, Wq, Wkv, Wo, bo)
    try:
        if "r" not in _runner_cache:
            _runner_cache["r"] = _CachedRunner(_get_nc(1))
        results = _runner_cache["r"].run(in_maps)
    except Exception:
        _runner_cache.pop("r", None)
        res = run_bass_kernel_spmd(_get_nc(1), in_maps, core_ids=list(range(8)))
        results = res.results
    out = np.empty((2, NK, F), dtype=np.float32)
    for c in range(8):
        b, qi = c // 4, c % 4
        out[b, qi * Q:(qi + 1) * Q] = results[c]["out_t"]
    return out


# revision 14
# speedup vs baseline: 1.6817x; 1.3213x over previous
"""Trainium2 Bass kernel for nn_Attention_88441966559243.

Attention with additive bias [B,N,N] and per-key bool mask, fp32 in/out.
  B=2, N=2048, QD=1024, HEADS=16, DIM_HEAD=64.

Sharding: 8 cores = (batch b = core//4) x (query slice q0 = (core%4)*512).
Each core computes out[b, q0:q0+512, :] completely on-device; the host gather
is a pure concatenation. No collectives.

v3 design (bf16 compute path, fp32 bias path; engines balanced):
  - Host supplies x / xq / Wq*scale / Wkv / Wo in bf16; bias/mask in fp32.
    Matmul accumulation is fp32 in PSUM, so CLT keeps bf16 operand error
    ~0.6% absolute on sim; bias is never quantized before exp (tail-safe);
    EB = exp(biasT+mask) is quantized bf16 AFTER exp (flat 0.4% rel).
    Numpy check of this exact dataflow: relmax 6e-3 (gate 2e-2).
  - Weights pinned SBUF-resident across For_i iterations (8 MB DMA saved).
  - A: PE-transpose x/xq (bf16, 1cyc/row); project qT (SBUF), kT, v'=[v|1]
    to DRAM bf16; PSUM->SBUF copies alternate ACT/DVE.
  - B: EB[key,q] = exp(biasT + maskneg) in bf16, SBUF-resident.
  - C: per head-pair, both subheads' sim chunks land in one 2-bank
    [128,1024] PSUM tile; one wide ACT exp -> ef bf16; DVE 2x-mode multiply
    by EB -> eT; av matmuls lagged 4 chunks.  Denominator via ones column,
    one PE-replication matmul pair + one reciprocal per head-pair.
  - D: out = oPair @ Wo + bo, K=128 per pair, SBUF-resident.
  PSUM banks statically partitioned 2(A/B/D/tail) + 4(simC) + 2(U) = 8 so
  successive For_i iterations can overlap.
"""
import sys
for _p in ("/opt/trn_rl_repo", "/root/.axon_site/_ro/trn_rl_repo"):
    if _p not in sys.path:
        sys.path.insert(0, _p)

import numpy as np
import ml_dtypes

import concourse.bass as bass
import concourse.mybir as mybir
from concourse import bacc
from concourse.tile import TileContext
from concourse.masks import make_identity
from concourse.bass_utils import run_bass_kernel_spmd

F = 1024          # feature dim (QD == INNER)
NK = 2048         # keys (full sequence)
Q = 512           # queries per core
H = 16            # heads
D = 64            # head dim
DV = 65           # head dim + ones column
SCALE = D ** -0.5
MASK_NEG = -30000.0

FC = F // 128      # 8 feature chunks
KC = NK // 128     # 16 key chunks
NB = NK // 512     # 4 key 512-blocks

f32 = mybir.dt.float32
fr = mybir.dt.float32r
bt = mybir.dt.bfloat16
AF = mybir.ActivationFunctionType


def build_nc(niter: int = 1, STAGES: str = "ABCD"):
    nc = bacc.Bacc(None, target_bir_lowering=False)

    x_in = nc.dram_tensor("x_in", [NK, F], bt, kind="ExternalInput")
    xq_in = nc.dram_tensor("xq_in", [Q, F], bt, kind="ExternalInput")
    bias_in = nc.dram_tensor("bias_in", [Q, NK], f32, kind="ExternalInput")
    maskneg_in = nc.dram_tensor("maskneg_in", [128, KC], f32, kind="ExternalInput")
    wq_in = nc.dram_tensor("wq_in", [F, F], bt, kind="ExternalInput")
    wkv_in = nc.dram_tensor("wkv_in", [F, 2 * F], bt, kind="ExternalInput")
    wo_in = nc.dram_tensor("wo_in", [F, F], bt, kind="ExternalInput")
    bo_in = nc.dram_tensor("bo_in", [1, F], fr, kind="ExternalInput")
    out_t = nc.dram_tensor("out_t", [Q, F], f32, kind="ExternalOutput")

    with TileContext(nc) as tc:
        with (
            tc.tile_pool(name="const", bufs=1) as constp,
            tc.tile_pool(name="dram", bufs=1, space="DRAM") as dramp,
            tc.tile_pool(name="psAB", bufs=2, space="PSUM") as psAB,
            tc.tile_pool(name="psC", bufs=2, space="PSUM") as psCp,
            tc.tile_pool(name="psu", bufs=2, space="PSUM") as psUp,
        ):
            # ---- constants & pinned weights ----
            ident = constp.tile([128, 128], f32)
            make_identity(nc, ident)
            ident_b = constp.tile([128, 128], bt)
            nc.scalar.copy(ident_b[:, :], ident[:, :])
            ones_f = constp.tile([128, 128], f32)
            nc.vector.memset(ones_f[:, :], 1.0)
            ones_r = constp.tile([128, 128], fr)
            nc.scalar.copy(ones_r[:, :], ones_f[:, :])
            ones_b = constp.tile([128, 128], bt)
            nc.scalar.copy(ones_b[:, :], ones_f[:, :])
            masksb = constp.tile([128, KC], f32)
            nc.sync.dma_start(masksb[:, :], maskneg_in[:, :])
            bo_sb = constp.tile([1, F], fr)
            nc.sync.dma_start(bo_sb[:, :], bo_in[:, :])
            bo_rep = constp.tile([128, F], f32)

            wq = [constp.tile([128, F], bt, name=f"wq{i}") for i in range(FC)]
            wk = [constp.tile([128, F], bt, name=f"wk{i}") for i in range(FC)]
            wv = [constp.tile([128, F], bt, name=f"wv{i}") for i in range(FC)]
            wo = [constp.tile([128, F], bt, name=f"wo{i}") for i in range(H // 2)]
            for fc in range(FC):
                nc.sync.dma_start(wq[fc][:, :],
                                  wq_in[fc * 128:(fc + 1) * 128, :])
                nc.sync.dma_start(wk[fc][:, :],
                                  wkv_in[fc * 128:(fc + 1) * 128, 0:F])
                nc.sync.dma_start(wv[fc][:, :],
                                  wkv_in[fc * 128:(fc + 1) * 128, F:2 * F])
            for i in range(H // 2):
                nc.sync.dma_start(wo[i][:, :], wo_in[i * 128:(i + 1) * 128, :])

            # bo broadcast to 128 partitions, once
            for nb2 in range(2):
                ps = psAB.tile([128, 512], f32, tag="psab", name="psab")
                nc.tensor.matmul(ps[:, :], ones_r[0:1, 0:128],
                                 bo_sb[0:1, nb2 * 512:(nb2 + 1) * 512],
                                 start=True, stop=True)
                nc.scalar.copy(bo_rep[:, nb2 * 512:(nb2 + 1) * 512], ps[:, :])

            vprime = dramp.tile([NK, H * DV], bt)       # v' (keys-major)
            kTd = dramp.tile([F, NK], bt)               # k^T

            def body(_iv=None):
                with (
                    tc.tile_pool(name="qTp", bufs=1) as qTp,
                    tc.tile_pool(name="EBp", bufs=1) as EBp,
                    tc.tile_pool(name="oPp", bufs=1) as oPp,
                ):
                    qT = [qTp.tile([128, Q], bt, tag=f"qT{i}", name=f"qT{i}")
                          for i in range(FC)]
                    EB = [EBp.tile([128, Q], bt, tag=f"EB{i}", name=f"EB{i}")
                          for i in range(KC)]
                    oPair = [oPp.tile([128, Q], bt, tag=f"oP{i}", name=f"oP{i}")
                             for i in range(H // 2)]

                    # ======== stage A ========
                    with (
                        tc.tile_pool(name="xTp", bufs=1) as xTp,
                        tc.tile_pool(name="kst", bufs=3) as kstp,
                    ):
                        xT = [xTp.tile([128, NK], bt, tag=f"xT{i}", name=f"xT{i}")
                              for i in range(FC)]

                        cp_flip = [0]

                        def cpy(dst, src):
                            # alternate PSUM->SBUF copies between ACT and DVE
                            if cp_flip[0] % 2 == 0:
                                nc.scalar.copy(dst, src)
                            else:
                                nc.vector.tensor_copy(dst, src)
                            cp_flip[0] += 1

                        with tc.tile_pool(name="xqTp", bufs=1) as xqTp:
                            xqT = [xqTp.tile([128, Q], bt, tag=f"xqT{i}",
                                             name=f"xqT{i}") for i in range(FC)]
                            # ---- A1: transpose x -> xT, xq -> xqT ----
                            with tc.tile_pool(name="xn", bufs=5) as xnp:
                                for rg in range(4):
                                    xns = []
                                    for r4 in range(4):
                                        rc = rg * 4 + r4
                                        xn = xnp.tile([128, F], bt, name="xn")
                                        nc.sync.dma_start(
                                            xn[:, :],
                                            x_in[rc * 128:(rc + 1) * 128, :])
                                        xns.append(xn)
                                    for fc in range(FC):
                                        ps = psAB.tile([128, 512], bt, tag="psab",
                                                       name="psab_t")
                                        for r4 in range(4):
                                            nc.tensor.transpose(
                                                ps[:, r4 * 128:(r4 + 1) * 128],
                                                xns[r4][:, fc * 128:(fc + 1) * 128],
                                                ident_b[:, :])
                                        cpy(xT[fc][:, rg * 512:(rg + 1) * 512],
                                            ps[:, :])
                                xqs = []
                                for r4 in range(4):
                                    xn = xnp.tile([128, F], bt, name="xn")
                                    nc.sync.dma_start(
                                        xn[:, :], xq_in[r4 * 128:(r4 + 1) * 128, :])
                                    xqs.append(xn)
                                for fc in range(FC):
                                    ps = psAB.tile([128, 512], bt, tag="psab",
                                                   name="psab_t")
                                    for r4 in range(4):
                                        nc.tensor.transpose(
                                            ps[:, r4 * 128:(r4 + 1) * 128],
                                            xqs[r4][:, fc * 128:(fc + 1) * 128],
                                            ident_b[:, :])
                                    cpy(xqT[fc][:, :], ps[:, :])

                            # ---- A2: qT = (Wq*scale)^T @ xqT ----
                            for m in range(FC):
                                ps = psAB.tile([128, 512], f32, tag="psab", name="psab")
                                for fc in range(FC):
                                    nc.tensor.matmul(
                                        ps[:, :],
                                        wq[fc][:, m * 128:(m + 1) * 128],
                                        xqT[fc][:, :],
                                        start=(fc == 0), stop=(fc == FC - 1))
                                cpy(qT[m][:, :], ps[:, :])

                        # ---- A3: kT = Wk^T @ xT -> DRAM ----
                        for m in range(FC):
                            kst = kstp.tile([128, NK], bt, name="kst")
                            for nb in range(NB):
                                ps = psAB.tile([128, 512], f32, tag="psab", name="psab")
                                for fc in range(FC):
                                    nc.tensor.matmul(
                                        ps[:, :],
                                        wk[fc][:, m * 128:(m + 1) * 128],
                                        xT[fc][:, nb * 512:(nb + 1) * 512],
                                        start=(fc == 0), stop=(fc == FC - 1))
                                cpy(kst[:, nb * 512:(nb + 1) * 512], ps[:, :])
                            nc.sync.dma_start(kTd[m * 128:(m + 1) * 128, :],
                                              kst[:, :])

                        # ---- A4: v' = [x @ Wv | 1] -> DRAM ----
                        with tc.tile_pool(name="vst", bufs=3) as vstp:
                            for kc in range(KC):
                                vst = vstp.tile([128, H * DV], bt, name="vst")
                                for half in range(2):
                                    ps = psAB.tile([128, 512], f32, tag="psab", name="psab")
                                    for fc in range(FC):
                                        nc.tensor.matmul(
                                            ps[:, :],
                                            xT[fc][:, kc * 128:(kc + 1) * 128],
                                            wv[fc][:, half * 512:(half + 1) * 512],
                                            start=(fc == 0), stop=(fc == FC - 1))
                                    dst = vst[:, half * 8 * DV:(half + 1) * 8 * DV] \
                                        .rearrange("p (h x) -> p h x", x=DV)[:, :, 0:64]
                                    cpy(dst,
                                        ps[:, :].rearrange("p (h d) -> p h d", d=64))
                                ones_dst = vst[:, :].rearrange(
                                    "p (h x) -> p h x", x=DV)[:, :, 64:65]
                                nc.gpsimd.tensor_copy(
                                    ones_dst,
                                    ones_b[:, 0:H].rearrange("p (a b) -> p a b", b=1))
                                nc.sync.dma_start(
                                    vprime[kc * 128:(kc + 1) * 128, :], vst[:, :])

                    if "B" not in STAGES:
                        with tc.tile_pool(name="dbg", bufs=2) as dbgp:
                            dbg = dbgp.tile([128, 512], bt, name="dbg")
                            nc.sync.dma_start(dbg[:, :], kTd[0:128, 0:512])
                            dbf = dbgp.tile([128, 512], f32, name="dbf")
                            nc.vector.tensor_copy(dbf[:, :], dbg[:, :])
                            nc.sync.dma_start(out_t[0:128, 0:512], dbf[:, :])
                        return

                    # ---- B: EB = exp(bias^T + maskneg), bf16 ----
                    with tc.tile_pool(name="bn", bufs=4) as bnp:
                        bns = []
                        for qc in range(4):
                            bn = bnp.tile([128, NK], f32, name="bn")
                            nc.sync.dma_start(
                                bn[:, :], bias_in[qc * 128:(qc + 1) * 128, :])
                            bns.append(bn)
                        for kc in range(KC):
                            ps = psAB.tile([128, 512], f32, tag="psab", name="psab")
                            for qc in range(4):
                                nc.tensor.transpose(
                                    ps[:, qc * 128:(qc + 1) * 128],
                                    bns[qc][:, kc * 128:(kc + 1) * 128],
                                    ident[:, :])
                            nc.scalar.activation(
                                EB[kc][:, :], ps[:, :], AF.Exp,
                                bias=masksb[:, kc:kc + 1], scale=1.0)

                    if "C" not in STAGES:
                        with tc.tile_pool(name="dbg", bufs=2) as dbgp:
                            dbf = dbgp.tile([128, 512], f32, name="dbf")
                            nc.vector.tensor_copy(dbf[:, :], EB[0][:, :])
                            nc.sync.dma_start(out_t[0:128, 0:512], dbf[:, :])
                        return

                    # ---- C: attention, head pairs ----
                    with (
                        tc.tile_pool(name="vph", bufs=2) as vphp,
                        tc.tile_pool(name="kph", bufs=2) as kphp,
                        tc.tile_pool(name="ef", bufs=3) as efp,
                        tc.tile_pool(name="et", bufs=6) as ep,
                        tc.tile_pool(name="dsb", bufs=2) as dsbp,
                        tc.tile_pool(name="rrep", bufs=2) as rrepp,
                    ):
                        for hp in range(H // 2):
                            vph = vphp.tile([128, KC * 2 * DV], bt, name="vph")
                            nc.sync.dma_start(
                                vph[:, :].rearrange("p (kc d) -> p kc d",
                                                    d=2 * DV),
                                vprime[:, 2 * hp * DV:(2 * hp + 2) * DV]
                                .rearrange("(kc p) d -> p kc d", p=128))
                            kph = kphp.tile([128, NK], bt, name="kph")
                            nc.sync.dma_start(
                                kph[:, :], kTd[hp * 128:(hp + 1) * 128, :])
                            psU2 = [psUp.tile([DV, 512], f32, name="psu")
                                    for _ in range(2)]
                            pending = []

                            def drain_av(upto):
                                while pending and pending[0][0] <= upto:
                                    kc0, eT0 = pending.pop(0)
                                    for sub in range(2):
                                        nc.tensor.matmul(
                                            psU2[sub][:, :],
                                            vph[:, kc0 * 2 * DV + sub * DV:
                                                kc0 * 2 * DV + (sub + 1) * DV],
                                            eT0[:, sub * 512:(sub + 1) * 512],
                                            start=(kc0 == 0),
                                            stop=(kc0 == KC - 1))

                            for kc in range(KC):
                                ps = psCp.tile([128, 1024], f32, name="psc")
                                for sub in range(2):
                                    po = sub * 64
                                    nc.tensor.matmul(
                                        ps[:, sub * 512:(sub + 1) * 512],
                                        kph[po:po + 64, kc * 128:(kc + 1) * 128],
                                        qT[hp][po:po + 64, :],
                                        start=True, stop=True)
                                ef = efp.tile([128, 1024], bt, name="ef")
                                nc.scalar.activation(
                                    ef[:, :], ps[:, :], AF.Exp,
                                    bias=masksb[:, kc:kc + 1], scale=1.0)
                                eT = ep.tile([128, 1024], bt, name="eT")
                                for sub in range(2):
                                    nc.vector.tensor_mul(
                                        eT[:, sub * 512:(sub + 1) * 512],
                                        ef[:, sub * 512:(sub + 1) * 512],
                                        EB[kc][:, :])
                                pending.append((kc, eT))
                                drain_av(kc - 4)
                            drain_av(KC)

                            # tail: denominators, reciprocal, normalize
                            for sub in range(2):
                                Dsb = dsbp.tile([DV, 512], fr, name="Dsb")
                                nc.scalar.copy(Dsb[64:65, :],
                                               psU2[sub][64:65, :])
                                psR = psAB.tile([128, 512], f32, tag="psab",
                                                name="psab")
                                nc.tensor.matmul(psR[0:64, :],
                                                 ones_r[64:65, 0:64],
                                                 Dsb[64:65, :],
                                                 start=True, stop=True)
                                rrep = rrepp.tile([64, 512], f32, name="rrep")
                                nc.vector.reciprocal_approx_fast(
                                    out=rrep[:, :], in_=psR[0:64, :])
                                nc.vector.tensor_mul(
                                    oPair[hp][sub * 64:(sub + 1) * 64, :],
                                    psU2[sub][0:64, :], rrep[:, :])

                    if "D" not in STAGES:
                        with tc.tile_pool(name="dbg", bufs=2) as dbgp:
                            dbf = dbgp.tile([128, 512], f32, name="dbf")
                            nc.vector.tensor_copy(dbf[:, :], oPair[0][:, :])
                            nc.sync.dma_start(out_t[0:128, 0:512], dbf[:, :])
                        return

                    # ======== stage D ========
                    with tc.tile_pool(name="fin", bufs=3) as finp:
                        for mc in range(4):
                            for nb2 in range(2):
                                psF = psAB.tile([128, 512], f32, tag="psab", name="psab")
                                for hp in range(H // 2):
                                    nc.tensor.matmul(
                                        psF[:, :],
                                        oPair[hp][:, mc * 128:(mc + 1) * 128],
                                        wo[hp][:, nb2 * 512:(nb2 + 1) * 512],
                                        start=(hp == 0),
                                        stop=(hp == H // 2 - 1))
                                fin = finp.tile([128, 512], f32, name="fin")
                                nc.vector.tensor_add(
                                    fin[:, :], psF[:, :],
                                    bo_rep[:, nb2 * 512:(nb2 + 1) * 512])
                                nc.sync.dma_start(
                                    out_t[mc * 128:(mc + 1) * 128,
                                          nb2 * 512:(nb2 + 1) * 512],
                                    fin[:, :])

            if niter == 1:
                body()
            else:
                with tc.For_i(0, niter, 1) as iv:
                    body(iv)

    nc.finalize()
    return nc


_nc_cache = {}


def _get_nc(niter=1):
    if niter not in _nc_cache:
        _nc_cache[niter] = build_nc(niter)
    return _nc_cache[niter]


def make_in_maps(x, bias, mask, Wq, Wkv, Wo, bo):
    bf16 = ml_dtypes.bfloat16
    x = np.asarray(x, dtype=np.float32)
    bias = np.asarray(bias, dtype=np.float32)
    mask = np.asarray(mask)
    x_b = x.astype(bf16)
    wq_b = (np.asarray(Wq, dtype=np.float32) * SCALE).astype(bf16)
    wkv_b = np.asarray(Wkv, dtype=np.float32).astype(bf16)
    wo_b = np.asarray(Wo, dtype=np.float32).astype(bf16)
    in_maps = []
    for c in range(8):
        b, qi = c // 4, c % 4
        q0 = qi * Q
        maskneg = np.where(mask[b], 0.0, MASK_NEG).astype(np.float32)
        in_maps.append({
            "x_in": np.ascontiguousarray(x_b[b]),
            "xq_in": np.ascontiguousarray(x_b[b, q0:q0 + Q]),
            "bias_in": np.ascontiguousarray(bias[b, q0:q0 + Q]),
            "maskneg_in": np.ascontiguousarray(maskneg.reshape(KC, 128).T),
            "wq_in": wq_b,
            "wkv_in": wkv_b,
            "wo_in": wo_b,
            "bo_in": np.ascontiguousarray(
                np.asarray(bo, dtype=np.float32).reshape(1, F)),
        })
    return in_maps


class _CachedRunner:
    """Jit the NEFF-backed executable once; repeat kernel() calls then skip
    the ~40s relower/recompile and run in ~0.1s."""

    def __init__(self, nc, n_cores=8):
        import jax
        from jax.sharding import Mesh, PartitionSpec
        from jax.experimental.shard_map import shard_map
        from concourse.bass2jax import (_bass_exec_p, install_neuronx_cc_hook,
                                        partition_id_tensor)
        install_neuronx_cc_hook()
        self.jax = jax
        self.n_cores = n_cores
        pname = nc.partition_id_tensor.name if nc.partition_id_tensor else None
        in_names, out_names, out_avals, zeros = [], [], [], []
        for alloc in nc.m.functions[0].allocations:
            if not isinstance(alloc, mybir.MemoryLocationSet):
                continue
            name = alloc.memorylocations[0].name
            if alloc.kind == "ExternalInput":
                if name != pname:
                    in_names.append(name)
            elif alloc.kind == "ExternalOutput":
                out_names.append(name)
                shape = tuple(alloc.tensor_shape)
                dt_np = mybir.dt.np(alloc.dtype)
                out_avals.append(jax.core.ShapedArray(shape, dt_np))
                zeros.append(np.zeros(shape, dt_np))
        self.in_names, self.out_names = in_names, out_names
        self.out_avals, self.zeros = out_avals, zeros
        all_names = in_names + out_names + ([pname] if pname else [])

        def _body(*args):
            ops = list(args)
            if pname is not None:
                ops.append(partition_id_tensor())
            return tuple(_bass_exec_p.bind(
                *ops, out_avals=tuple(out_avals), in_names=tuple(all_names),
                out_names=tuple(out_names), lowering_input_output_aliases=(),
                sim_require_finite=True, sim_require_nnan=True, nc=nc))

        mesh = Mesh(np.asarray(jax.devices()[:n_cores]), ("core",))
        spec_in = (PartitionSpec("core"),) * (len(in_names) + len(out_names))
        spec_out = (PartitionSpec("core"),) * len(out_names)
        self.fn = jax.jit(shard_map(_body, mesh=mesh, in_specs=spec_in,
                                    out_specs=spec_out, check_rep=False),
                          keep_unused=True)

    def run(self, in_maps):
        n = self.n_cores
        args = [np.concatenate([np.asarray(in_maps[c][k]) for c in range(n)], axis=0)
                for k in self.in_names]
        args += [np.zeros((n * z.shape[0], *z.shape[1:]), z.dtype)
                 for z in self.zeros]
        outs = self.fn(*args)
        self.jax.block_until_ready(outs)
        return [{k: np.asarray(outs[i]).reshape(n, *self.out_avals[i].shape)[c]
                 for i, k in enumerate(self.out_names)} for c in range(n)]


_runner_cache = {}


def kernel(x, bias, mask, Wq, Wkv, Wo, bo):
    in_maps = make_in_maps(x, bias, mask, Wq, Wkv, Wo, bo)
    try:
        if "r" not in _runner_cache:
            _runner_cache["r"] = _CachedRunner(_get_nc(1))
        results = _runner_cache["r"].run(in_maps)
    except Exception:
        _runner_cache.pop("r", None)
        res = run_bass_kernel_spmd(_get_nc(1), in_maps, core_ids=list(range(8)))
        results = res.results
    out = np.empty((2, NK, F), dtype=np.float32)
    for c in range(8):
        b, qi = c // 4, c % 4
        out[b, qi * Q:(qi + 1) * Q] = results[c]["out_t"]
    return out


# revision 19
# speedup vs baseline: 1.9017x; 1.1308x over previous
"""Trainium2 Bass kernel for nn_Attention_88441966559243.

Attention with additive bias [B,N,N] and per-key bool mask, fp32 in/out.
  B=2, N=2048, QD=1024, HEADS=16, DIM_HEAD=64.

Sharding: 8 cores = (batch b = core//4) x (query slice q0 = (core%4)*512).
Each core computes out[b, q0:q0+512, :] completely on-device; the host gather
is a pure concatenation. No collectives.

v4 design (serial-critical-path focused: For_i has an all-engine barrier at
the back-edge, so per-iteration latency is what's measured):
  - bf16 compute path everywhere except the bias (fp32 until exp; EB=exp(
    biasT+mask) quantized bf16 AFTER exp). Numpy model: relmax 6e-3 (gate 2e-2).
  - No DRAM scratch at all: kT chunks stay in SBUF (kst tiles) and v' is
    SBUF-resident, so per-iteration DMA is just inputs+outputs (~17 MB).
  - Emission order pipelines the serial chain: [x/xq/bias/w DMAs] -> A1
    transposes -> B (bias transpose + EB exp on ACT, hidden under A4/A2 PE)
    -> A4 v' -> A2 qT -> then per head-pair: kT m-chunk (PE) interleaved
    with that pair's attention block, so kT production hides under the
    ACT-bound exp stream.  PSUM->SBUF copies balanced across ACT/DVE.
  - C: both subheads' sim chunks in one 2-bank [128,1024] PSUM tile; one
    wide ACT exp -> ef bf16; DVE 2x multiply by EB -> eT; av matmuls lag 4
    chunks.  Denominator via v' ones column + PE replication + DVE
    reciprocal; normalized straight into packed oPair tiles.
  - D: out = oPair @ Wo + bo (wo pinned in SBUF across iterations).
"""
import sys
for _p in ("/opt/trn_rl_repo", "/root/.axon_site/_ro/trn_rl_repo"):
    if _p not in sys.path:
        sys.path.insert(0, _p)

import numpy as np
import ml_dtypes

import concourse.bass as bass
import concourse.mybir as mybir
from concourse import bacc
from concourse.tile import TileContext
from concourse.masks import make_identity
from concourse.bass_utils import run_bass_kernel_spmd

F = 1024          # feature dim (QD == INNER)
NK = 2048         # keys (full sequence)
Q = 512           # queries per core
H = 16            # heads
D = 64            # head dim
DV = 65           # head dim + ones column
SCALE = D ** -0.5
MASK_NEG = -30000.0

FC = F // 128      # 8 feature chunks
KC = NK // 128     # 16 key chunks
NB = NK // 512     # 4 key 512-blocks

f32 = mybir.dt.float32
fr = mybir.dt.float32r
bt = mybir.dt.bfloat16
AF = mybir.ActivationFunctionType


def build_nc(niter: int = 1, STAGES: str = "ABCD"):
    nc = bacc.Bacc(None, target_bir_lowering=False)

    x_in = nc.dram_tensor("x_in", [NK, F], bt, kind="ExternalInput")
    xq_in = nc.dram_tensor("xq_in", [Q, F], bt, kind="ExternalInput")
    bias_in = nc.dram_tensor("bias_in", [Q, NK], f32, kind="ExternalInput")
    maskneg_in = nc.dram_tensor("maskneg_in", [128, KC], f32, kind="ExternalInput")
    wq_in = nc.dram_tensor("wq_in", [F, F], bt, kind="ExternalInput")
    wkv_in = nc.dram_tensor("wkv_in", [F, 2 * F], bt, kind="ExternalInput")
    wo_in = nc.dram_tensor("wo_in", [F, F], bt, kind="ExternalInput")
    bo_in = nc.dram_tensor("bo_in", [1, F], fr, kind="ExternalInput")
    out_t = nc.dram_tensor("out_t", [Q, F], f32, kind="ExternalOutput")

    with TileContext(nc) as tc:
        with (
            tc.tile_pool(name="const", bufs=1) as constp,
            tc.tile_pool(name="psAB", bufs=2, space="PSUM") as psAB,
            tc.tile_pool(name="psC", bufs=2, space="PSUM") as psCp,
            tc.tile_pool(name="psu", bufs=2, space="PSUM") as psUp,
        ):
            # ---- constants & pinned wo ----
            ident = constp.tile([128, 128], f32)
            make_identity(nc, ident)
            ident_b = constp.tile([128, 128], bt)
            nc.scalar.copy(ident_b[:, :], ident[:, :])
            ones_f = constp.tile([128, 128], f32)
            nc.vector.memset(ones_f[:, :], 1.0)
            ones_r = constp.tile([128, 128], fr)
            nc.scalar.copy(ones_r[:, :], ones_f[:, :])
            ones_b = constp.tile([128, 128], bt)
            nc.scalar.copy(ones_b[:, :], ones_f[:, :])
            masksb = constp.tile([128, KC], f32)
            nc.sync.dma_start(masksb[:, :], maskneg_in[:, :])
            bo_sb = constp.tile([1, F], fr)
            nc.sync.dma_start(bo_sb[:, :], bo_in[:, :])
            bo_rep = constp.tile([128, F], f32)

            wo = [constp.tile([128, F], bt, name=f"wo{i}") for i in range(H // 2)]
            for i in range(H // 2):
                nc.sync.dma_start(wo[i][:, :], wo_in[i * 128:(i + 1) * 128, :])

            # bo broadcast to 128 partitions, once
            for nb2 in range(2):
                ps = psAB.tile([128, 512], f32, tag="psab", name="psab")
                nc.tensor.matmul(ps[:, :], ones_r[0:1, 0:128],
                                 bo_sb[0:1, nb2 * 512:(nb2 + 1) * 512],
                                 start=True, stop=True)
                nc.scalar.copy(bo_rep[:, nb2 * 512:(nb2 + 1) * 512], ps[:, :])

            def body(_iv=None):
                with (
                    tc.tile_pool(name="qTp", bufs=1) as qTp,
                    tc.tile_pool(name="EBp", bufs=1) as EBp,
                    tc.tile_pool(name="oPp", bufs=1) as oPp,
                    tc.tile_pool(name="vSp", bufs=1) as vSp,
                    tc.tile_pool(name="wload", bufs=16) as wlp,
                ):
                    qT = [qTp.tile([128, Q], bt, tag=f"qT{i}", name=f"qT{i}")
                          for i in range(FC)]
                    EB = [EBp.tile([128, Q], bt, tag=f"EB{i}", name=f"EB{i}")
                          for i in range(KC)]
                    oPair = [oPp.tile([128, Q], bt, tag=f"oP{i}", name=f"oP{i}")
                             for i in range(H // 2)]
                    vSB = [vSp.tile([128, H * DV], bt, tag=f"vS{i}",
                                    name=f"vS{i}") for i in range(KC)]

                    cp_flip = [0]

                    def cpy(dst, src, eng=None):
                        if eng is None:
                            eng = cp_flip[0] % 2
                            cp_flip[0] += 1
                        if eng == 0:
                            nc.scalar.copy(dst, src)
                        else:
                            nc.vector.tensor_copy(dst, src)

                    with (
                        tc.tile_pool(name="xTp", bufs=1) as xTp,
                        tc.tile_pool(name="kst", bufs=3) as kstp,
                        tc.tile_pool(name="bn", bufs=4) as bnp,
                    ):
                        xT = [xTp.tile([128, NK], bt, tag=f"xT{i}", name=f"xT{i}")
                              for i in range(FC)]

                        # ---- input DMA issue order: x, xq, bias, wv, wq ----
                        xns_all = []
                        with tc.tile_pool(name="xn", bufs=8) as xnp:
                            for rc in range(16):
                                xn = xnp.tile([128, F], bt, name="xn", tag="xn")
                                nc.sync.dma_start(
                                    xn[:, :], x_in[rc * 128:(rc + 1) * 128, :])
                                xns_all.append(xn)
                            xqs = []
                            for r4 in range(4):
                                xn = xnp.tile([128, F], bt, name="xn", tag="xn")
                                nc.sync.dma_start(
                                    xn[:, :], xq_in[r4 * 128:(r4 + 1) * 128, :])
                                xqs.append(xn)
                            bnsh = []
                            for qc in range(4):
                                bn = bnp.tile([128, F], f32, name="bn", tag="bn")
                                nc.sync.dma_start(
                                    bn[:, :],
                                    bias_in[qc * 128:(qc + 1) * 128, 0:F])
                                bnsh.append(bn)
                            wv = [wlp.tile([128, F], bt, tag="w", name="w")
                                  for _ in range(FC)]
                            for fc in range(FC):
                                nc.sync.dma_start(
                                    wv[fc][:, :],
                                    wkv_in[fc * 128:(fc + 1) * 128, F:2 * F])
                            wq = [wlp.tile([128, F], bt, tag="w", name="w")
                                  for _ in range(FC)]
                            for fc in range(FC):
                                nc.sync.dma_start(
                                    wq[fc][:, :],
                                    wq_in[fc * 128:(fc + 1) * 128, :])

                            # ---- A1: transpose x -> xT, xq -> xqT ----
                            with tc.tile_pool(name="xqTp", bufs=1) as xqTp:
                                xqT = [xqTp.tile([128, Q], bt, tag=f"xqT{i}",
                                                 name=f"xqT{i}")
                                       for i in range(FC)]
                                for rg in range(4):
                                    for fc in range(FC):
                                        ps = psAB.tile([128, 512], bt,
                                                       tag="psab", name="psab_t")
                                        for r4 in range(4):
                                            nc.tensor.transpose(
                                                ps[:, r4 * 128:(r4 + 1) * 128],
                                                xns_all[rg * 4 + r4]
                                                [:, fc * 128:(fc + 1) * 128],
                                                ident_b[:, :])
                                        cpy(xT[fc][:, rg * 512:(rg + 1) * 512],
                                            ps[:, :])
                                for fc in range(FC):
                                    ps = psAB.tile([128, 512], bt,
                                                   tag="psab", name="psab_t")
                                    for r4 in range(4):
                                        nc.tensor.transpose(
                                            ps[:, r4 * 128:(r4 + 1) * 128],
                                            xqs[r4][:, fc * 128:(fc + 1) * 128],
                                            ident_b[:, :])
                                    cpy(xqT[fc][:, :], ps[:, :])

                                # ---- B: EB = exp(bias^T + mask) (ACT),
                                #      hidden under A4/A2 PE work; bias
                                #      loaded in column halves to fit SBUF ----
                                for half in range(2):
                                    if half == 1:
                                        bnsh = []
                                        for qc in range(4):
                                            bn = bnp.tile([128, F], f32,
                                                          name="bn", tag="bn")
                                            nc.sync.dma_start(
                                                bn[:, :],
                                                bias_in[qc * 128:(qc + 1) * 128,
                                                        F:2 * F])
                                            bnsh.append(bn)
                                    for kc in range(half * 8, half * 8 + 8):
                                        kcl = kc - half * 8
                                        ps = psAB.tile([128, 512], f32,
                                                       tag="psab", name="psab")
                                        for qc in range(4):
                                            nc.tensor.transpose(
                                                ps[:, qc * 128:(qc + 1) * 128],
                                                bnsh[qc][:, kcl * 128:
                                                         (kcl + 1) * 128],
                                                ident[:, :])
                                        nc.scalar.activation(
                                            EB[kc][:, :], ps[:, :], AF.Exp,
                                            bias=masksb[:, kc:kc + 1], scale=1.0)

                                # ---- A4: v' = [x @ Wv | 1] -> SBUF ----
                                for kc in range(KC):
                                    for half in range(2):
                                        ps = psAB.tile([128, 512], f32,
                                                       tag="psab", name="psab")
                                        for fc in range(FC):
                                            nc.tensor.matmul(
                                                ps[:, :],
                                                xT[fc][:, kc * 128:(kc + 1) * 128],
                                                wv[fc][:, half * 512:(half + 1) * 512],
                                                start=(fc == 0),
                                                stop=(fc == FC - 1))
                                        dst = vSB[kc][:, half * 8 * DV:
                                                      (half + 1) * 8 * DV] \
                                            .rearrange("p (h x) -> p h x",
                                                       x=DV)[:, :, 0:64]
                                        cpy(dst,
                                            ps[:, :].rearrange(
                                                "p (h d) -> p h d", d=64))
                                    ones_dst = vSB[kc][:, :].rearrange(
                                        "p (h x) -> p h x", x=DV)[:, :, 64:65]
                                    nc.gpsimd.tensor_copy(
                                        ones_dst,
                                        ones_b[:, 0:H].rearrange(
                                            "p (a b) -> p a b", b=1))

                                # ---- A2: qT = (Wq*scale)^T @ xqT ----
                                for m in range(FC):
                                    ps = psAB.tile([128, 512], f32,
                                                   tag="psab", name="psab")
                                    for fc in range(FC):
                                        nc.tensor.matmul(
                                            ps[:, :],
                                            wq[fc][:, m * 128:(m + 1) * 128],
                                            xqT[fc][:, :],
                                            start=(fc == 0), stop=(fc == FC - 1))
                                    cpy(qT[m][:, :], ps[:, :])

                        # wk reuses the wq/wv slots (consumers done)
                        wk = [wlp.tile([128, F], bt, tag="w", name="w")
                              for _ in range(FC)]
                        for fc in range(FC):
                            nc.sync.dma_start(
                                wk[fc][:, :], wkv_in[fc * 128:(fc + 1) * 128, 0:F])

                        if "C" not in STAGES:
                            with tc.tile_pool(name="dbg", bufs=2) as dbgp:
                                dbf = dbgp.tile([128, 512], f32, name="dbf")
                                nc.vector.tensor_copy(dbf[:, :], EB[0][:, :])
                                nc.sync.dma_start(out_t[0:128, 0:512], dbf[:, :])
                            return

                        # ---- C (+A3 interleaved): attention per head pair ----
                        with (
                            tc.tile_pool(name="ef", bufs=3) as efp,
                            tc.tile_pool(name="et", bufs=5) as ep,
                            tc.tile_pool(name="dsb", bufs=2) as dsbp,
                            tc.tile_pool(name="rrep", bufs=2) as rrepp,
                        ):
                            for hp in range(H // 2):
                                # A3 slice: kT rows for this head pair (SBUF)
                                kst = kstp.tile([128, NK], bt, name="kst")
                                for nb in range(NB):
                                    ps = psAB.tile([128, 512], f32,
                                                   tag="psab", name="psab")
                                    for fc in range(FC):
                                        nc.tensor.matmul(
                                            ps[:, :],
                                            wk[fc][:, hp * 128:(hp + 1) * 128],
                                            xT[fc][:, nb * 512:(nb + 1) * 512],
                                            start=(fc == 0), stop=(fc == FC - 1))
                                    cpy(kst[:, nb * 512:(nb + 1) * 512],
                                        ps[:, :], eng=1)

                                psU2 = [psUp.tile([DV, 512], f32, name="psu")
                                        for _ in range(2)]
                                pending = []

                                def drain_av(upto):
                                    while pending and pending[0][0] <= upto:
                                        kc0, eT0 = pending.pop(0)
                                        for sub in range(2):
                                            nc.tensor.matmul(
                                                psU2[sub][:, :],
                                                vSB[kc0][:, (2 * hp + sub) * DV:
                                                         (2 * hp + sub + 1) * DV],
                                                eT0[:, sub * 512:(sub + 1) * 512],
                                                start=(kc0 == 0),
                                                stop=(kc0 == KC - 1))

                                for kc in range(KC):
                                    ps = psCp.tile([128, 1024], f32, name="psc")
                                    for sub in range(2):
                                        po = sub * 64
                                        nc.tensor.matmul(
                                            ps[:, sub * 512:(sub + 1) * 512],
                                            kst[po:po + 64,
                                                kc * 128:(kc + 1) * 128],
                                            qT[hp][po:po + 64, :],
                                            start=True, stop=True)
                                    ef = efp.tile([128, 1024], bt, name="ef")
                                    nc.scalar.activation(
                                        ef[:, :], ps[:, :], AF.Exp,
                                        scale=1.0)
                                    eT = ep.tile([128, 1024], bt, name="eT")
                                    for sub in range(2):
                                        nc.vector.tensor_mul(
                                            eT[:, sub * 512:(sub + 1) * 512],
                                            ef[:, sub * 512:(sub + 1) * 512],
                                            EB[kc][:, :])
                                    pending.append((kc, eT))
                                    drain_av(kc - 4)
                                drain_av(KC)

                                # tail: denominators, reciprocal, normalize
                                for sub in range(2):
                                    Dsb = dsbp.tile([DV, 512], fr, name="Dsb")
                                    nc.vector.tensor_copy(Dsb[64:65, :],
                                                          psU2[sub][64:65, :])
                                    psR = psAB.tile([128, 512], f32,
                                                    tag="psab", name="psab")
                                    nc.tensor.matmul(psR[0:64, :],
                                                     ones_r[64:65, 0:64],
                                                     Dsb[64:65, :],
                                                     start=True, stop=True)
                                    rrep = rrepp.tile([64, 512], f32,
                                                      name="rrep")
                                    nc.vector.reciprocal_approx_fast(
                                        out=rrep[:, :], in_=psR[0:64, :])
                                    nc.vector.tensor_mul(
                                        oPair[hp][sub * 64:(sub + 1) * 64, :],
                                        psU2[sub][0:64, :], rrep[:, :])

                    if "D" not in STAGES:
                        with tc.tile_pool(name="dbg", bufs=2) as dbgp:
                            dbf = dbgp.tile([128, 512], f32, name="dbf")
                            nc.vector.tensor_copy(dbf[:, :], oPair[0][:, :])
                            nc.sync.dma_start(out_t[0:128, 0:512], dbf[:, :])
                        return

                    # ======== stage D ========
                    with tc.tile_pool(name="fin", bufs=3) as finp:
                        for mc in range(4):
                            for nb2 in range(2):
                                psF = psAB.tile([128, 512], f32,
                                                tag="psab", name="psab")
                                for hp in range(H // 2):
                                    nc.tensor.matmul(
                                        psF[:, :],
                                        oPair[hp][:, mc * 128:(mc + 1) * 128],
                                        wo[hp][:, nb2 * 512:(nb2 + 1) * 512],
                                        start=(hp == 0),
                                        stop=(hp == H // 2 - 1))
                                fin = finp.tile([128, 512], f32, name="fin")
                                nc.vector.tensor_add(
                                    fin[:, :], psF[:, :],
                                    bo_rep[:, nb2 * 512:(nb2 + 1) * 512])
                                nc.sync.dma_start(
                                    out_t[mc * 128:(mc + 1) * 128,
                                          nb2 * 512:(nb2 + 1) * 512],
                                    fin[:, :])

            if niter == 1:
                body()
            else:
                with tc.For_i(0, niter, 1) as iv:
                    body(iv)

    nc.finalize()
    return nc


_nc_cache = {}


def _get_nc(niter=1):
    if niter not in _nc_cache:
        _nc_cache[niter] = build_nc(niter)
    return _nc_cache[niter]


def make_in_maps(x, bias, mask, Wq, Wkv, Wo, bo):
    bf16 = ml_dtypes.bfloat16
    x = np.asarray(x, dtype=np.float32)
    bias = np.asarray(bias, dtype=np.float32)
    mask = np.asarray(mask)
    x_b = x.astype(bf16)
    wq_b = (np.asarray(Wq, dtype=np.float32) * SCALE).astype(bf16)
    wkv_b = np.asarray(Wkv, dtype=np.float32).astype(bf16)
    wo_b = np.asarray(Wo, dtype=np.float32).astype(bf16)
    in_maps = []
    for c in range(8):
        b, qi = c // 4, c % 4
        q0 = qi * Q
        maskneg = np.where(mask[b], 0.0, MASK_NEG).astype(np.float32)
        in_maps.append({
            "x_in": np.ascontiguousarray(x_b[b]),
            "xq_in": np.ascontiguousarray(x_b[b, q0:q0 + Q]),
            "bias_in": np.ascontiguousarray(bias[b, q0:q0 + Q]),
            "maskneg_in": np.ascontiguousarray(maskneg.reshape(KC, 128).T),
            "wq_in": wq_b,
            "wkv_in": wkv_b,
            "wo_in": wo_b,
            "bo_in": np.ascontiguousarray(
                np.asarray(bo, dtype=np.float32).reshape(1, F)),
        })
    return in_maps


class _CachedRunner:
    """Jit the NEFF-backed executable once; repeat kernel() calls then skip
    the ~40s relower/recompile and run in ~0.1s."""

    def __init__(self, nc, n_cores=8):
        import jax
        from jax.sharding import Mesh, PartitionSpec
        from jax.experimental.shard_map import shard_map
        from concourse.bass2jax import (_bass_exec_p, install_neuronx_cc_hook,
                                        partition_id_tensor)
        install_neuronx_cc_hook()
        self.jax = jax
        self.n_cores = n_cores
        pname = nc.partition_id_tensor.name if nc.partition_id_tensor else None
        in_names, out_names, out_avals, zeros = [], [], [], []
        for alloc in nc.m.functions[0].allocations:
            if not isinstance(alloc, mybir.MemoryLocationSet):
                continue
            name = alloc.memorylocations[0].name
            if alloc.kind == "ExternalInput":
                if name != pname:
                    in_names.append(name)
            elif alloc.kind == "ExternalOutput":
                out_names.append(name)
                shape = tuple(alloc.tensor_shape)
                dt_np = mybir.dt.np(alloc.dtype)
                out_avals.append(jax.core.ShapedArray(shape, dt_np))
                zeros.append(np.zeros(shape, dt_np))
        self.in_names, self.out_names = in_names, out_names
        self.out_avals, self.zeros = out_avals, zeros
        all_names = in_names + out_names + ([pname] if pname else [])

        def _body(*args):
            ops = list(args)
            if pname is not None:
                ops.append(partition_id_tensor())
            return tuple(_bass_exec_p.bind(
                *ops, out_avals=tuple(out_avals), in_names=tuple(all_names),
                out_names=tuple(out_names), lowering_input_output_aliases=(),
                sim_require_finite=True, sim_require_nnan=True, nc=nc))

        mesh = Mesh(np.asarray(jax.devices()[:n_cores]), ("core",))
        spec_in = (PartitionSpec("core"),) * (len(in_names) + len(out_names))
        spec_out = (PartitionSpec("core"),) * len(out_names)
        self.fn = jax.jit(shard_map(_body, mesh=mesh, in_specs=spec_in,
                                    out_specs=spec_out, check_rep=False),
                          keep_unused=True)

    def run(self, in_maps):
        n = self.n_cores
        args = [np.concatenate([np.asarray(in_maps[c][k]) for c in range(n)], axis=0)
                for k in self.in_names]
        args += [np.zeros((n * z.shape[0], *z.shape[1:]), z.dtype)
                 for z in self.zeros]
        outs = self.fn(*args)
        self.jax.block_until_ready(outs)
        return [{k: np.asarray(outs[i]).reshape(n, *self.out_avals[i].shape)[c]
                 for i, k in enumerate(self.out_names)} for c in range(n)]


_runner_cache = {}


def kernel(x, bias, mask, Wq, Wkv, Wo, bo):
    in_maps = make_in_maps(x, bias, mask, Wq, Wkv, Wo, bo)
    try:
        if "r" not in _runner_cache:
            _runner_cache["r"] = _CachedRunner(_get_nc(1))
        results = _runner_cache["r"].run(in_maps)
    except Exception:
        _runner_cache.pop("r", None)
        res = run_bass_kernel_spmd(_get_nc(1), in_maps, core_ids=list(range(8)))
        results = res.results
    out = np.empty((2, NK, F), dtype=np.float32)
    for c in range(8):
        b, qi = c // 4, c % 4
        out[b, qi * Q:(qi + 1) * Q] = results[c]["out_t"]
    return out
